# revision 65
# baseline (speedup 1.0000x reference)
"""CRF decoder loss kernel for Trainium2 (Bass/Tile), 8-core data parallel.

Algorithm notes
---------------
The CRF forward algorithm is computed in the "hot" (exp) domain:
    u_{t+1}[j,b] = el_t[j,b] * sum_i exp(T[j,i]) * u_t[i,b]
with el_t = exp(logit_t + bias - C0).  The batch is split into two
independent 8-sequence half-chains running in antiphase: each step is
a PE matmul (stationary exp(T)^T, 52x8) + a DVE elementwise multiply
(52x8) per chain.  The DVE cost is dominated by the fixed PSUM access,
so two narrow chains have a lower per-step latency (~526ns vs ~535ns)
than one 16-wide chain while overlapping on the engines.  Everything
else (emissions, rescales, gold score, norm-score selection) is
scheduled into the idle engine gaps of the chains, sized to fit the
~260ns DVE windows (SBUF-only pieces go to the idle Pool engine).

A constant e^{-C0} per step keeps magnitudes near 1; every R steps a
data-dependent rescale (divide by the state-mass sum, accumulate log)
bounds fp32 range.  The rescale's multiply is folded into the el column
consumed DEF steps later, and only the raw reciprocal is stored per
event (in a flat partition-0 row -- engine writes must start at a
32-aligned partition); the logs are batched into one Ln near the end
of the scan (keeps Exp the only in-scan ACT function -- no
activation-table swaps on the critical path).

State row 51 (END) has zero incoming weights in exp(T)^T (transition
from END is -100), so it carries the "end-dot" sum_i exp(T[END,i])
u_t[i] forward one step -- the norm-score numerator for every prefix
length.  Host-built one-hot selects (from `lens`) pick each sequence's
prefix.  End-dot rows are gathered into a [128, 4*BL] tile by
transposing DMAs issued as soon as each 128-slice block completes; the
last NTAIL+1 prefixes are instead produced by tiny [1,BL] matmuls
(stationary column exp(T)[END,:]) into a partition-0 PSUM stage, so
the tail after the last scan step needs no DMA round-trip.  Per-block
max/Ln/select run in late-scan gaps.

Emission logits are produced in 16-step chunks with float32r matmuls,
one chunk ahead of the scan (chunk 0 split in two halves to start the
scan as early as possible).  The scan is truncated at T = max(lens)
(program compiled per distinct T).

Sharding: pure data parallel over batch (16 sequences/core); per-core
partial losses summed on host.
"""

import numpy as np
from contextlib import ExitStack

import concourse.bass as bass
import concourse.tile as tile
from concourse import bacc
from concourse import mybir
from concourse.bass_utils import run_bass_kernel_spmd

F32 = mybir.dt.float32
F32R = mybir.dt.float32r
AF = mybir.ActivationFunctionType
ALU = mybir.AluOpType

B, S, D = 128, 512, 1024
L = 50            # real labels
NL = L + 2        # + START, END
START, END = 50, 51
NCORES = 8
BL = B // NCORES  # 16 sequences per core
TCH = 16          # timesteps per emission chunk
KD = D // 128     # contraction chunks for emission matmul
R = 16            # rescale period (steps)
DEF = 4           # rescale apply deferral (steps)
C0 = 7.5          # constant per-step log damping folded into emission bias
NTAIL = 6         # trailing prefixes whose end-dots come from stage matmuls

# packed-constants column layout (single DMA); flat-row parts depend on T
PK_TT = 0         # [52, 52]  T^T
PK_TM = 52        # [52, 52]  T
PK_CNT = 104      # [52, 52]  pair counts
PK_B = 156        # [50, 1]   bias
PK_CL = 157       # [50, 1]   label counts
PK_SE = 158       # [128, 64] end-dot select (lens <= T-NTAIL-1)
PK_U0 = 222       # [NL, BL]  u_0 one-hot
PK_S2 = 238       # [NTAIL+1, BL] 2D stage select
PK_FL = 254       # [1, FW]   flat select row (late DMA; read mid-scan)


def n_events(T):
    """Rescale events: at t = R*k + R-1, needing el column t+DEF <= T-1."""
    k = 0
    while R * k + R - 1 + DEF <= T - 1:
        k += 1
    return k


def flat_w(T):
    """Flat row: [NEV*BL recips][BL ones][(NTAIL+1)*BL stage end-dots]."""
    return (n_events(T) + 1 + NTAIL + 1) * BL


def pk_w(T):
    return PK_FL + flat_w(T)


def build_program(T):
    NCHUNK = -(-T // TCH)
    assert 3 <= NCHUNK and NCHUNK * TCH <= S
    NEV = n_events(T)
    HTB = TCH * BL // 2  # half-chunk column count (128)
    HSP = 8 * BL         # chunk-0 emission split point
    FW = flat_w(T)
    LNW = (NEV + 1) * BL       # flat cols Ln'd mid-scan (recips + ones pad)
    STO = (NEV + 1) * BL       # stage block offset in the flat row
    PKW = pk_w(T)

    nc = bacc.Bacc("TRN2", target_bir_lowering=False, debug=False,
                   num_devices=NCORES)

    xT_d = nc.dram_tensor("xT", [KD, 128, S * BL], F32R, kind="ExternalInput")
    WT_d = nc.dram_tensor("WT", [KD, 128, L], F32R, kind="ExternalInput")
    OH_d = nc.dram_tensor("OH", [L, S * BL], F32, kind="ExternalInput")
    EL_d = nc.dram_tensor("ELI", [2, S * BL], F32, kind="ExternalInput")
    PK_d = nc.dram_tensor("PK", [128, PKW], F32, kind="ExternalInput")
    loss_d = nc.dram_tensor("loss", [1, BL + 1], F32, kind="ExternalOutput")

    with tile.TileContext(nc) as tc, ExitStack() as ctx:
        consts = ctx.enter_context(tc.tile_pool(name="consts", bufs=1))
        xpool = ctx.enter_context(tc.tile_pool(name="xpool", bufs=3))
        ohpool = ctx.enter_context(tc.tile_pool(name="ohpool", bufs=3))
        smalls = ctx.enter_context(tc.tile_pool(name="smalls", bufs=2))
        lgp = ctx.enter_context(tc.tile_pool(name="lgp", bufs=2, space="PSUM"))
        lgp0 = ctx.enter_context(tc.tile_pool(name="lgp0", bufs=1, space="PSUM"))
        pp = ctx.enter_context(tc.tile_pool(name="pp", bufs=1, space="PSUM"))
        miscp = ctx.enter_context(tc.tile_pool(name="miscp", bufs=1, space="PSUM"))

        # ------------- input DMAs (HWDGE + transfer are serialized; order
        # and granularity chosen so the chunk-0 emission starts earliest) ---
        pk = consts.tile([128, PKW], F32, name="pk")
        # scan-critical constants land in a small first transfer; the
        # flat select row (read mid-scan around t~=tsc) ships after all
        # the head DMAs
        nc.sync.dma_start(out=pk[:, 0:PK_FL], in_=PK_d.ap()[:, 0:PK_FL])
        wt = consts.tile([128, KD * L], F32R, name="wt")
        nc.sync.dma_start(
            out=wt[:, :],
            in_=WT_d.ap()[:, :, :].rearrange("k p l -> p k l"))

        xt_tiles = {}
        oh_tiles = {}

        def xt_dma(ch, half=None, k=None):
            if ch not in xt_tiles:
                xt_tiles[ch] = xpool.tile([128, KD * TCH * BL], F32R,
                                          name="xt", tag="xt")
            c0 = ch * TCH * BL
            if k is not None:
                # one contraction-block pair: paces its matmuls on the
                # serialized HWDGE queue (avoids ready-burst clogging of
                # the PE queue ahead of released scan matmuls)
                h0 = 0 if half is None else half * HSP
                w = TCH * BL if half is None else (
                    HSP if half == 0 else TCH * BL - HSP)
                xt3 = xt_tiles[ch][:, :].rearrange("p (k q) -> p k q", k=KD,
                                                   q=TCH * BL)
                nc.sync.dma_start(
                    out=xt3[:, k:k + 2, h0:h0 + w],
                    in_=xT_d.ap()[k:k + 2, :, c0 + h0:c0 + h0 + w]
                    .rearrange("k p x -> p k x"))
                return
            xt3 = xt_tiles[ch][:, :].rearrange("p (k q) -> p k q", k=KD,
                                               q=TCH * BL)
            if half is None:
                nc.sync.dma_start(
                    out=xt3,
                    in_=xT_d.ap()[:, :, c0:c0 + TCH * BL]
                    .rearrange("k p x -> p k x"))
            else:
                h0 = half * HSP
                w = HSP if half == 0 else TCH * BL - HSP
                nc.sync.dma_start(
                    out=xt3[:, :, h0:h0 + w],
                    in_=xT_d.ap()[:, :, c0 + h0:c0 + h0 + w]
                    .rearrange("k p x -> p k x"))

        def oh_dma(ch):
            oh = ohpool.tile([L, TCH * BL], F32, name="oh", tag="oh")
            nc.sync.dma_start(out=oh[:, :],
                              in_=OH_d.ap()[:, ch * TCH * BL:(ch + 1) * TCH * BL])
            oh_tiles[ch] = oh

        # all chunk-0/1 input as per-k-pair DMAs: small transfers let the
        # half-A matmuls start on the first pair, and the HWDGE pacing
        # keeps later matmuls from bursting ahead of the scan chain.
        for k in range(0, KD, 2):
            xt_dma(0, half=0, k=k)

        # ---------------- big state buffers ----------------
        el_buf = consts.tile([NL, S * BL], F32, name="el_buf")
        u_buf = consts.tile([NL, (T + 1) * BL], F32, name="u_buf")
        uacc = consts.tile([L, 2 * NCHUNK], F32, name="uacc")
        scratch = consts.tile([NL, TCH * BL], F32, name="scratch")
        flat = consts.tile([1, FW], F32, name="flat")
        flln = consts.tile([1, FW], F32, name="flln")
        endbuf = consts.tile([128, 4 * BL], F32, name="endbuf")
        endlog = consts.tile([128, 4 * BL], F32, name="endlog")
        ns1 = consts.tile([1, BL], F32, name="ns1")
        ns2 = consts.tile([1, BL], F32, name="ns2")
        ns3 = consts.tile([1, BL], F32, name="ns3")

        nc.sync.dma_start(out=el_buf[START:START + 2, :], in_=EL_d.ap()[:, :])
        for k in range(0, KD, 2):
            xt_dma(0, half=1, k=k)
        for k in range(0, KD, 2):
            xt_dma(1, k=k)
        oh_dma(0)
        oh_dma(1)
        nc.sync.dma_start(out=pk[:, PK_FL:PKW], in_=PK_d.ap()[:, PK_FL:PKW])

        # ---------------- derived constants ----------------
        # Pin the combined exp+ln activation table once; the auto-inserted
        # per-function loads otherwise thrash 1.3us swaps into the scan.
        nc.scalar.add_instruction(mybir.InstLoadActFuncSet(
            name=nc.scalar.bass.get_next_instruction_name(),
            act_func_set_id=6, ins=[], outs=[]))
        stat = consts.tile([NL, NL], F32, name="stat")  # stat[i,j] = exp(T[j,i])
        nc.scalar.activation(out=stat[:, :], in_=pk[0:NL, PK_TT:PK_TT + NL],
                             func=AF.Exp)
        btile = consts.tile([L, 1], F32, name="btile")
        nc.vector.tensor_scalar_add(btile[:, :], pk[0:L, PK_B:PK_B + 1], -C0)
        ones = consts.tile([128, 1], F32, name="ones")
        nc.vector.memset(ones[:, :], 1.0)
        ones_r = consts.tile([1, NL], F32, name="ones_r")
        nc.vector.memset(ones_r[:, :], 1.0)
        eps = consts.tile([128, 1], F32, name="eps")
        nc.vector.memset(eps[:, :], 1e-38)


        # u_0 = e_START comes packed (engine writes need 32-aligned
        # partition starts, so no single-row memset at partition 50)
        nc.vector.tensor_copy(u_buf[:, 0:BL], pk[0:NL, PK_U0:PK_U0 + BL])
        nc.vector.memset(flln[:, NEV * BL:(NEV + 1) * BL], 1.0)
        # blk-3 endbuf rows beyond the DMA'd range: 1.0 (log 0, masked)
        r_dma = T - NTAIL - 385  # rows 0..r_dma-1 come from the late DMA
        r_al = (r_dma // 32) * 32
        nc.vector.memset(endbuf[r_al:128, 3 * BL:4 * BL], 1.0)

        # ---------------- emission helpers ----------------
        lg_tiles = {}

        def em_mm(ch, k, half=None):
            key = (ch, half)
            h0 = 0 if half is None else half * HSP
            w = TCH * BL if half is None else (
                HSP if half == 0 else TCH * BL - HSP)
            if k == 0:
                if half is None:
                    lg_tiles[key] = lgp.tile([L, w], F32, name="lg", tag="lg")[:, :]
                else:
                    if "h" not in lg_tiles:
                        lg_tiles["h"] = lgp0.tile([L, TCH * BL], F32,
                                                  name="lgh", tag="lgh")
                    lg_tiles[key] = lg_tiles["h"][:, h0:h0 + w]
            nc.tensor.matmul(
                lg_tiles[key],
                lhsT=wt[:, k * L:(k + 1) * L],
                rhs=xt_tiles[ch][:, k * TCH * BL + h0:k * TCH * BL + h0 + w],
                start=(k == 0), stop=(k == KD - 1))

        def em_exp(ch, half=None):
            c0 = ch * TCH * BL + (0 if half is None else half * HSP)
            w = TCH * BL if half is None else (
                HSP if half == 0 else TCH * BL - HSP)
            nc.scalar.activation(out=el_buf[0:L, c0:c0 + w],
                                 in_=lg_tiles[(ch, half)],
                                 func=AF.Exp, bias=btile[:, 0:1], scale=1.0)

        QTB = TCH * BL // 4

        def em_unary(ch, piece):
            # quarters + half-reduces: each op fits the DVE idle window
            # between the two half-chain Hadamards of a scan step
            if piece < 4:
                sl = slice(piece * QTB, (piece + 1) * QTB)
                if ch == 0:
                    lg_ap = lg_tiles["h"][:, sl]
                else:
                    lg_ap = lg_tiles[(ch, None)][:, sl]
                nc.vector.tensor_mul(scratch[0:L, sl], lg_ap,
                                     oh_tiles[ch][:, sl])
            else:
                # SBUF-only: runs on the otherwise-idle Pool engine so it
                # never collides with the scan Hadamards on DVE
                j = piece - 4
                nc.vector.tensor_reduce(
                    out=uacc[:, 2 * ch + j:2 * ch + j + 1],
                    in_=scratch[0:L, j * HTB:(j + 1) * HTB],
                    axis=mybir.AxisListType.X, op=ALU.add)

        # all of chunk 0 + chunk 1 emission upfront: the per-k DMAs pace
        # these matmuls through the pre-scan window one at a time
        for k in range(KD):
            em_mm(0, k, half=0)
        em_exp(0, half=0)
        for k in range(KD):
            em_mm(0, k, half=1)
        em_exp(0, half=1)
        for k in range(KD):
            em_mm(1, k)
        em_exp(1)

        def endbuf_dma(blk, s_hi):
            """Gather end-dot slices [blk*128+2 .. s_hi] into endbuf."""
            s_lo = blk * 128 + 2
            n = s_hi - s_lo + 1
            src = u_buf[END:END + 1, s_lo * BL:(s_hi + 1) * BL]
            nc.sync.dma_start(
                out=endbuf[0:n, blk * BL:(blk + 1) * BL],
                in_=src.rearrange("p (q b) -> p q b", q=n, b=BL))

        def end_block_ops(blk, kind):
            cs = slice(blk * BL, (blk + 1) * BL)
            if kind == 1:
                # bias keeps Ln finite for underflowed/padded end-dots
                # (inputs are >= 0, so this equals Ln(max(x, 1e-38)))
                nc.scalar.activation(out=endlog[:, cs], in_=endbuf[:, cs],
                                     func=AF.Ln, bias=eps[:, 0:1], scale=1.0)
            else:
                nc.vector.tensor_mul(endlog[:, cs], endlog[:, cs],
                                     pk[:, PK_SE + blk * BL:PK_SE + (blk + 1) * BL])

        res = miscp.tile([1, BL + 1], F32, name="res", tag="m1")
        resn = res[:, 0:BL]
        stg = miscp.tile([NTAIL + 1, 2 * BL], F32, name="stg", tag="m3")
        ps_ap = stg[0:1, BL:2 * BL]
        st_ln = consts.tile([NTAIL + 1, BL], F32, name="st_ln")
        # one-hot-column stationaries: LZ[:, j*(NTAIL+1)+j] = exp(T)[END,:],
        # so accumulating matmuls land end-dot(T-NTAIL+j) on PARTITION j of
        # the stage tile; the final reduction then rides the nacc matmul's
        # partition contraction instead of a free-dim reduce.
        LZ = consts.tile([NL, (NTAIL + 1) * (NTAIL + 1)], F32, name="LZ")
        nc.vector.memset(LZ[:, :], 0.0)
        for j in range(NTAIL + 1):
            nc.vector.tensor_copy(
                LZ[:, j * (NTAIL + 1) + j:j * (NTAIL + 1) + j + 1],
                stat[:, END:END + 1])

        def end_block_mm(blk, start):
            nc.tensor.matmul(resn, lhsT=ones[:, :],
                             rhs=endlog[:, blk * BL:(blk + 1) * BL],
                             start=start, stop=False)

        # flat-row ops, split in halves so each fits a DVE scan gap
        KH = (NEV + 1) // 2

        def scl_flat_ops(kind, half=0):
            h0 = 0 if half == 0 else KH * BL
            kh = KH if half == 0 else NEV + 1 - KH
            hs = slice(h0, h0 + kh * BL)
            if kind == 0 and NEV > 0:      # batched Ln of the reciprocals
                nc.scalar.activation(out=flln[:, 0:NEV * BL],
                                     in_=flat[:, 0:NEV * BL], func=AF.Ln)
            elif kind == 1:                # select/mask multiply (Pool)
                nc.gpsimd.tensor_mul(flln[:, hs], flln[:, hs],
                                     pk[0:1, PK_FL + h0:PK_FL + h0 + kh * BL])
            elif kind == 2:                # reduce blocks -> [1, BL]
                dst = ns1 if half == 0 else ns2
                nc.vector.tensor_reduce(
                    out=dst[:, :],
                    in_=flln[:, hs].rearrange("p (k b) -> p b k",
                                              k=kh, b=BL),
                    axis=mybir.AxisListType.X, op=ALU.add)
            else:                          # accumulate into the result
                src = ns1 if half == 0 else ns2
                nc.tensor.matmul(resn, lhsT=ones[0:1, :], rhs=src[:, :],
                                 start=False, stop=False)

        # late-scan schedule: endbuf blocks 0..2, then the flat scale row.
        # All after the last in-scan em_exp so the single Exp->Ln table
        # swap lands in an ACT gap.  Falls back to the tail for small T.
        tproc = (NCHUNK - 2) * TCH + 10
        last_rec_t = R * (NEV - 1) + R if NEV > 0 else 0
        tsc = max(tproc + 9, last_rec_t + 2)
        mid_blocks = tproc + 8 <= T - 1 and T - 1 >= 386
        mid_scl = mid_blocks and tsc + 7 <= T - 1
        dve_sched = {}
        act_sched = {}
        pe_sched = {}
        if mid_blocks:
            for blk in range(3):
                act_sched[tproc + blk] = (end_block_ops, blk, 1)      # Ln
                dve_sched[tproc + 2 + blk] = (end_block_ops, blk, 2)  # sel mul
                pe_sched[tproc + 5 + blk] = blk                    # resn matmul
        if mid_scl:
            act_sched[tsc] = (scl_flat_ops, 0)           # flat Ln
            dve_sched[tsc + 2] = (scl_flat_ops, 1, 0)    # flat mul halves
            dve_sched[tsc + 3] = (scl_flat_ops, 1, 1)
            dve_sched[tsc + 4] = (scl_flat_ops, 2, 0)    # flat reduce halves
            dve_sched[tsc + 5] = (scl_flat_ops, 2, 1)
            pe_sched[tsc + 6] = -1                       # resn matmuls
            pe_sched[tsc + 7] = -2
        t_dma3 = T - NTAIL  # iteration that DMAs blk-3 slices 386..T-NTAIL

        # ---------------- scan with interleaved emission ----------------
        pend_ps = None      # event step whose mass-sum matmul is due
        pend_pb = None      # (t_ev, rec_ap) whose broadcast matmul is due
        pend_premul = None  # (t_apply, pb) el pre-multiply due
        kev_ctr = 0

        for t in range(T):
            ch, tl = divmod(t, TCH)

            # PE: scan matmuls for step t, split into two independent
            # half-batch chains (the DVE Hadamard cost is dominated by the
            # fixed PSUM access, so two 8-wide chains in antiphase run at a
            # lower per-step latency than one 16-wide chain)
            HB = BL // 2
            pa = pp.tile([NL, HB], F32, name="pa", tag="pA")
            pb2 = pp.tile([NL, HB], F32, name="pb2", tag="pB")
            nc.tensor.matmul(pa[:, :], lhsT=stat[:, :],
                             rhs=u_buf[:, t * BL:t * BL + HB],
                             start=True, stop=True)
            nc.tensor.matmul(pb2[:, :], lhsT=stat[:, :],
                             rhs=u_buf[:, t * BL + HB:(t + 1) * BL],
                             start=True, stop=True)

            # stage end-dots for the last NTAIL prefixes: end-dot(t) is
            # row END of stat^T u_t, recomputed with the one-hot-column
            # stationary so it lands on partition j of the stage tile
            if T - NTAIL <= t:
                j = t - (T - NTAIL)
                lz = LZ[:, j * (NTAIL + 1):(j + 1) * (NTAIL + 1)]
                nc.tensor.matmul(stg[0:NTAIL + 1, 0:HB], lhsT=lz,
                                 rhs=u_buf[:, t * BL:t * BL + HB],
                                 start=(j == 0), stop=False)
                nc.tensor.matmul(stg[0:NTAIL + 1, HB:BL], lhsT=lz,
                                 rhs=u_buf[:, t * BL + HB:(t + 1) * BL],
                                 start=(j == 0), stop=False)

            # rescale event bookkeeping (all off the serial chain)
            rec_now = None
            if pend_ps is not None:
                nc.tensor.matmul(ps_ap, lhsT=ones[0:L, :],
                                 rhs=u_buf[0:L, t * BL:(t + 1) * BL],
                                 start=True, stop=True)
                rec_now = (pend_ps, kev_ctr)
                kev_ctr += 1
                pend_ps = None
            if pend_pb is not None:
                t_ev, rec_ap = pend_pb
                pb = miscp.tile([NL, BL], F32, name="pb", tag="m2")
                nc.tensor.matmul(pb[:, :], lhsT=ones_r[:, :], rhs=rec_ap,
                                 start=True, stop=True)
                pend_premul = (t_ev + DEF, pb)
                pend_pb = None
            if t in pe_sched:
                w = pe_sched[t]
                if w == -1:
                    scl_flat_ops(3, 0)
                elif w == -2:
                    scl_flat_ops(3, 1)
                else:
                    end_block_mm(w, start=(w == 0))

            # DVE: the serial Hadamards, one per half-batch chain
            nc.vector.tensor_mul(u_buf[:, (t + 1) * BL:(t + 1) * BL + HB],
                                 pa[:, :], el_buf[:, t * BL:t * BL + HB])
            nc.vector.tensor_mul(u_buf[:, (t + 1) * BL + HB:(t + 2) * BL],
                                 pb2[:, :],
                                 el_buf[:, t * BL + HB:(t + 1) * BL])

            # DVE followers (start after the Hadamard in queue order).
            # Only the raw reciprocal is stored per event; logs are batched
            # late in the scan (keeps Exp the only in-scan ACT function).
            if rec_now is not None:
                t_ev, kev = rec_now
                rec_ap = flat[0:1, kev * BL:(kev + 1) * BL]
                nc.vector.reciprocal(rec_ap, ps_ap)
                pend_pb = (t_ev, rec_ap)
                rec_now = None
            if pend_premul is not None and pend_premul[0] == t + 1:
                t_apply, pb = pend_premul
                csl = slice(t_apply * BL, (t_apply + 1) * BL)
                nc.vector.tensor_mul(el_buf[0:START, csl],
                                     el_buf[0:START, csl], pb[0:START, :])
                pend_premul = None

            if t % R == R - 1 and t + DEF <= T - 1:
                pend_ps = t

            # interleaved emission pipeline (chunks 0/1 were done upfront)
            if ch == 0:
                if tl == 0 and 2 < NCHUNK:
                    xt_dma(2)
                    oh_dma(2)
            else:
                if tl == 0 and ch + 2 < NCHUNK:
                    xt_dma(ch + 2)
                    oh_dma(ch + 2)
                if ch + 1 < NCHUNK:
                    # tl 4..11: keeps the PE queue clear of the rescale
                    # side-matmuls that land on tl 0..1 after each event
                    if 4 <= tl < 4 + KD:
                        em_mm(ch + 1, tl - 4)
                    elif tl == 4 + KD:
                        em_exp(ch + 1)
            # unary (gold) ops for chunk ch, in DVE idle gaps (shifted
            # early for the second-to-last chunk to dodge the endbuf
            # block-processing slots)
            if ch < NCHUNK - 1:
                u_slots = ((0, 1, 3, 5, 7, 9) if ch == NCHUNK - 2
                           else (5, 7, 9, 11, 13, 15))
                if tl in u_slots:
                    em_unary(ch, u_slots.index(tl))
            # end-dot gathers for completed 128-slice blocks + late blk 3
            if t >= 137 and (t - 137) % 128 == 0 and (t - 137) // 128 < 3:
                blk = (t - 137) // 128
                endbuf_dma(blk, blk * 128 + 129)
            if t == t_dma3 and mid_blocks:
                endbuf_dma(3, T - NTAIL)
            if t in dve_sched:
                f, *args = dve_sched[t]
                f(*args)
            if t in act_sched:
                f, *args = act_sched[t]
                f(*args)

        # ---------------- tail ----------------
        # stage end-dot for the full prefix length T
        lz = LZ[:, NTAIL * (NTAIL + 1):(NTAIL + 1) * (NTAIL + 1)]
        nc.tensor.matmul(stg[0:NTAIL + 1, 0:HB], lhsT=lz,
                         rhs=u_buf[:, T * BL:T * BL + HB],
                         start=False, stop=False)
        nc.tensor.matmul(stg[0:NTAIL + 1, HB:BL], lhsT=lz,
                         rhs=u_buf[:, T * BL + HB:(T + 1) * BL],
                         start=False, stop=True)
        nc.scalar.activation(out=st_ln[:, :], in_=stg[0:NTAIL + 1, 0:BL],
                             func=AF.Ln, bias=eps[0:NTAIL + 1, 0:1], scale=1.0)
        nc.vector.tensor_mul(st_ln[:, :], st_ln[:, :],
                             pk[0:NTAIL + 1, PK_S2:PK_S2 + BL])

        if not mid_blocks:
            endbuf_dma(3, T - NTAIL)
            for blk in range(3):
                if not (137 + 128 * blk <= T - 1):
                    endbuf_dma(blk, blk * 128 + 129)
                end_block_ops(blk, 1)
                end_block_ops(blk, 2)
        end_block_ops(3, 1)
        end_block_ops(3, 2)
        if not mid_blocks:
            for blk in range(3):
                end_block_mm(blk, start=(blk == 0))
        end_block_mm(3, start=False)
        if not mid_scl:
            scl_flat_ops(0)
            for half in (0, 1):
                scl_flat_ops(1, half)
                scl_flat_ops(2, half)
                scl_flat_ops(3, half)
        nc.tensor.matmul(resn, lhsT=ones[0:NTAIL + 1, :], rhs=st_ln[:, :],
                         start=False, stop=True)

        # ---------------- gold score ----------------
        for piece in range(6):
            em_unary(NCHUNK - 1, piece)
        gt1 = consts.tile([NL, 1], F32, name="gt1")
        nc.vector.tensor_mul(scratch[0:NL, 0:NL], pk[0:NL, PK_TM:PK_TM + NL],
                             pk[0:NL, PK_CNT:PK_CNT + NL])
        nc.vector.tensor_reduce(out=gt1[:, :], in_=scratch[0:NL, 0:NL],
                                axis=mybir.AxisListType.X, op=ALU.add)
        gt2 = consts.tile([L, 1], F32, name="gt2")
        nc.vector.tensor_mul(gt2[:, :], pk[0:L, PK_B:PK_B + 1],
                             pk[0:L, PK_CL:PK_CL + 1])
        ur = consts.tile([L, 1], F32, name="ur")
        nc.vector.tensor_reduce(out=ur[:, :], in_=uacc[:, :],
                                axis=mybir.AxisListType.X, op=ALU.add)
        resg = res[:, BL:BL + 1]
        nc.tensor.matmul(resg, lhsT=ones[0:NL, :], rhs=gt1[:, :],
                         start=True, stop=False)
        nc.tensor.matmul(resg, lhsT=ones[0:L, :], rhs=gt2[:, :],
                         start=False, stop=False)
        nc.tensor.matmul(resg, lhsT=ones[0:L, :], rhs=ur[:, :],
                         start=False, stop=True)

        # pack [norm_b(0..BL-1), gold_total] and ship
        lt = smalls.tile([1, BL + 1], F32, name="lt", tag="lt")
        nc.vector.tensor_copy(lt[:, :], res[:, :])
        nc.sync.dma_start(out=loss_d.ap()[:, :], in_=lt[:, :])

    nc.compile()
    return nc


def prep_inputs(inputs, W, b, transition, lens, labels, T=None):
    """Host-side sharding + index preprocessing. Returns per-core input maps."""
    x = np.ascontiguousarray(np.asarray(inputs, dtype=np.float32))
    W = np.asarray(W, dtype=np.float32)
    b = np.asarray(b, dtype=np.float32)
    Tm = np.asarray(transition, dtype=np.float32)
    lens = np.asarray(lens).astype(np.int64)
    labels = np.asarray(labels).astype(np.int64)
    if T is None:
        T = chain_len(lens)
    NEV = n_events(T)
    FW = flat_w(T)
    PKW = pk_w(T)

    WT = np.ascontiguousarray(W.T).reshape(KD, 128, L)

    # (B,S,D) -> (D,S,B) once, then per-core contiguous slices
    xt_all = np.ascontiguousarray(np.transpose(x, (2, 1, 0)))  # (D, S, B)

    in_maps = []
    for c in range(NCORES):
        bs = slice(c * BL, (c + 1) * BL)
        lens_c = lens[bs]
        labels_c = labels[bs]

        xT = np.ascontiguousarray(xt_all[:, :, bs]).reshape(KD, 128, S * BL)

        mask = np.arange(S)[:, None] < lens_c[None, :]        # (S, BL)
        lab_t = labels_c.T                                     # (S, BL)
        OH = (lab_t[None, :, :] == np.arange(L)[:, None, None]) & mask[None]
        OH = np.ascontiguousarray(OH.astype(np.float32).reshape(L, S * BL))

        # pair counts following the reference labels_ext construction
        ext = np.full((BL, S + 2), END, dtype=np.int64)
        ext[:, 0] = START
        ext[:, 1:S + 1] = labels_c
        valid = np.arange(S + 2)[None, :] < (lens_c + 1)[:, None]
        ext = np.where(valid, ext, END)
        CNT = np.zeros((NL, NL), dtype=np.float32)
        pmask = np.arange(S + 1)[None, :] < (lens_c + 1)[:, None]
        to_ = ext[:, 1:][pmask]
        fr_ = ext[:, :-1][pmask]
        np.add.at(CNT, (to_, fr_), 1.0)

        CNTL = np.zeros((L,), dtype=np.float32)
        msk = np.arange(S)[None, :] < lens_c[:, None]
        np.add.at(CNTL, labels_c[msk], 1.0)

        # end-dot selection: DMA'd blocks cover lens <= T-NTAIL-1; the
        # stage flat row covers lens in [T-NTAIL, T]
        SELEND = np.zeros((128, 4 * BL), dtype=np.float32)
        q = lens_c - 1
        dma_mask = lens_c <= T - NTAIL - 1
        SELEND[q[dma_mask] % 128,
               (q[dma_mask] // 128) * BL + np.arange(BL)[dma_mask]] = 1.0

        # flat select row: [NEV recip blocks: -mask][C0*lens][stage 1-hots]
        FLS = np.zeros((FW,), dtype=np.float32)
        for k in range(NEV):
            FLS[k * BL:(k + 1) * BL] = -(
                lens_c >= (R * k + R + DEF)).astype(np.float32)
        FLS[NEV * BL:(NEV + 1) * BL] = C0 * lens_c.astype(np.float32)
        for j in range(NTAIL + 1):
            sl = slice((NEV + 1 + j) * BL, (NEV + 2 + j) * BL)
            FLS[sl] = (lens_c == (T - NTAIL + j)).astype(np.float32)

        PKa = np.zeros((128, PKW), dtype=np.float32)
        PKa[0:NL, PK_TT:PK_TT + NL] = Tm.T
        PKa[0:NL, PK_TM:PK_TM + NL] = Tm
        PKa[0:NL, PK_CNT:PK_CNT + NL] = CNT
        PKa[0:L, PK_B] = b
        PKa[0:L, PK_CL] = CNTL
        PKa[:, PK_SE:PK_SE + 4 * BL] = SELEND
        PKa[0, PK_FL:PK_FL + FW] = FLS
        PKa[START, PK_U0:PK_U0 + BL] = 1.0
        for j in range(NTAIL + 1):
            PKa[j, PK_U0 + BL:PK_U0 + 2 * BL] = (
                lens_c == (T - NTAIL + j)).astype(np.float32)

        ELINIT = np.zeros((2, S * BL), dtype=np.float32)
        ELINIT[1, :] = 1.0

        in_maps.append({
            "xT": xT, "WT": WT, "OH": OH, "ELI": ELINIT, "PK": PKa,
        })
    return in_maps


def chain_len(lens):
    return max(int(np.max(lens)), 397)


_NC_CACHE = {}


def kernel(inputs, W, b, transition, lens, labels, _trace=False, _tmpdir=None):
    T = chain_len(np.asarray(lens))
    in_maps = prep_inputs(inputs, W, b, transition, lens, labels, T=T)
    if T not in _NC_CACHE:
        _NC_CACHE[T] = build_program(T)
    nc = _NC_CACHE[T]
    res = run_bass_kernel_spmd(nc, in_maps, list(range(NCORES)),
                               trace=_trace, tmpdir=_tmpdir)
    total = np.float64(0.0)
    for r in res.results:
        v = r["loss"]
        total += np.float64(v[0, 0:BL].sum()) - np.float64(v[0, BL])
    out = np.float32(total)
    if _trace:
        return out, res
    return out
